# revision 6
# baseline (speedup 1.0000x reference)
"""Multi-head attention forward on 8 Trainium2 NeuronCores.

Problem: B=8, N=1024 tokens, C=1024 channels, H=16 heads, hd=64.
Returns (out [B,N,C], attn [B,H,N,N]) matching the reference
    qkv = x @ w_qkv ; attn = softmax(q k^T / sqrt(hd)) ; out = (attn v) @ w_proj + b_proj

Sharding: pure data parallel, one batch element per core, no collectives.

Per-core layout strategy (everything transpose-free on device):
  - host sends xT = x[b].T                       [C, N]   bf16
  - qT,kT = (w_qkv[:, :2C]).T-style matmul:      lhsT=w_qkv tile, rhs=xT  -> qkT [2C, N]
  - v     = x @ w_qkv[:, 2C:]:                   lhsT=xT tile, rhs=wv     -> v   [N(keys), C]
  - S^T_h [keys, tokens] = kT_h.T-matmul:        lhsT=kT_h (K=64), rhs=qT_h
  - expS = Exp(S^T * hd^-0.5)  (ACT, bf16 out; no max subtraction, scores are O(5))
  - ctx^T_h [64, N] = v_h.T @ expS  (accumulated over key tiles); a col-packed M=1
    ones-matmul accumulates rowsum[tokens] into psum partition 64 for free
  - recip = Exp(-Ln(rowsum))  (both funcs in the natural_log_exp table set)
  - bcast recip to 128 partitions with a K=1 ones matmul; attn^T = expS * bcast (DVE
    bf16 2x mode), DMA out per key tile; ctx^T normalized by the same bcast
  - out^T = w_proj-tile.T @ ctx^T + b_proj (per-partition bias), DMA out
Host transposes attnT -> attn and outT -> out (pure data movement).
"""

import numpy as np

B, N, C, H, HD = 8, 1024, 1024, 16, 64
P = 128
KT = C // P          # 8 tiles of 128 along any C/N axis
SCALE = HD ** -0.5   # 0.125
NCORES = 8


def _build():
    import concourse.mybir as mybir
    import concourse.tile as tile
    from concourse import bacc

    F32 = mybir.dt.float32
    BF16 = mybir.dt.bfloat16
    AF = mybir.ActivationFunctionType

    nc = bacc.Bacc("TRN2", target_bir_lowering=False, debug=False,
                   enable_asserts=True)

    xT_d = nc.dram_tensor("xT", [C, N], BF16, kind="ExternalInput")
    wqk_d = nc.dram_tensor("wqk", [C, 2 * C], BF16, kind="ExternalInput")
    wv_d = nc.dram_tensor("wv", [C, C], BF16, kind="ExternalInput")
    wp_d = nc.dram_tensor("wp", [C, C], BF16, kind="ExternalInput")
    bp_d = nc.dram_tensor("bp", [P, KT], F32, kind="ExternalInput")
    attnT_d = nc.dram_tensor("attnT", [H, N, N], BF16, kind="ExternalOutput")
    outT_d = nc.dram_tensor("outT", [C, N], F32, kind="ExternalOutput")

    with tile.TileContext(nc) as tc:
        with tc.tile_pool(name="persist", bufs=1) as pp, \
             tc.tile_pool(name="psum", bufs=1, space="PSUM") as ps:

            # ---- persistent SBUF tensors
            qk_sb = pp.tile([P, 2 * KT, N], BF16)   # outer 0..7 = qT, 8..15 = kT
            # v with a ones-column appended per head: cols 0..63 = v, 64..65 = 1.0
            # (col 64 rides along in the ctx matmul to accumulate softmax rowsums)
            v_sb = pp.tile([P, KT, H, 66], BF16)
            wp_sb = pp.tile([P, KT, C], BF16)
            bp_sb = pp.tile([P, KT], F32)
            ctxT_sb = pp.tile([P, KT, N], BF16)
            ones_row = pp.tile([1, P], BF16)

            nc.vector.memset(v_sb[:, :, :, 64:66], 1.0)
            nc.vector.memset(ones_row[:], 1.0)
            nc.sync.dma_start(out=bp_sb[:], in_=bp_d.ap())
            for k in range(KT):
                nc.sync.dma_start(out=wp_sb[:, k, :],
                                  in_=wp_d.ap()[k * P:(k + 1) * P, :])

            with tc.tile_pool(name="ph1", bufs=1) as p1:
                xT_sb = p1.tile([P, KT, N], BF16)
                wqk_sb = p1.tile([P, KT, 2 * C], BF16)
                wv_sb = p1.tile([P, KT, C], BF16)
                for k in range(KT):
                    nc.sync.dma_start(out=xT_sb[:, k, :],
                                      in_=xT_d.ap()[k * P:(k + 1) * P, :])
                for mg in range(4):  # 512-col chunks so early m-tiles unblock fast
                    for k in range(KT):
                        nc.sync.dma_start(
                            out=wqk_sb[:, k, mg * 512:(mg + 1) * 512],
                            in_=wqk_d.ap()[k * P:(k + 1) * P, mg * 512:(mg + 1) * 512])
                for k in range(KT):
                    nc.sync.dma_start(out=wv_sb[:, k, :],
                                      in_=wv_d.ap()[k * P:(k + 1) * P, :])

                def qk_tile(m):
                    # qkT rows m*128..(m+1)*128 = (x @ w_qkv[:, :2C]).T tile
                    psA = ps.tile([P, N], F32, tag="s", bufs=2)
                    for k in range(KT):
                        for t in range(2):
                            nc.tensor.matmul(
                                psA[:, t * 512:(t + 1) * 512],
                                wqk_sb[:, k, m * P:(m + 1) * P],
                                xT_sb[:, k, t * 512:(t + 1) * 512],
                                start=(k == 0), stop=(k == KT - 1))
                    nc.vector.tensor_copy(out=qk_sb[:, m, :], in_=psA[:, :])

                def v_tile(m):
                    # v rows (keys) m*128..(m+1)*128 = (x @ w_qkv[:, 2C:]) tile
                    psA = ps.tile([P, N], F32, tag="s", bufs=2)
                    for k in range(KT):
                        for t in range(2):
                            nc.tensor.matmul(
                                psA[:, t * 512:(t + 1) * 512],
                                xT_sb[:, k, m * P:(m + 1) * P],
                                wv_sb[:, k, t * 512:(t + 1) * 512],
                                start=(k == 0), stop=(k == KT - 1))
                    nc.vector.tensor_copy(
                        out=v_sb[:, m, :, 0:64],
                        in_=psA[:, :].rearrange("p (h d) -> p h d", d=HD))

                with tc.tile_pool(name="ph2", bufs=1) as p2:

                    def head(h):
                        hb = (h % 2) * 64       # partition base of this head's dims
                        mq = h // 2             # qT outer index
                        mk = KT + h // 2        # kT outer index
                        expS = p2.tile([P, KT, N], BF16, tag="expS", bufs=2)
                        psC = ps.tile([P, N], F32, tag="c", bufs=1)
                        for i in range(KT):
                            psS = ps.tile([P, N], F32, tag="s", bufs=2)
                            for t in range(2):
                                nc.tensor.matmul(
                                    psS[:, t * 512:(t + 1) * 512],
                                    qk_sb[hb:hb + 64, mk, i * P:(i + 1) * P],
                                    qk_sb[hb:hb + 64, mq, t * 512:(t + 1) * 512],
                                    start=True, stop=True)
                            nc.scalar.activation(expS[:, i, :], psS[:, :],
                                                 AF.Exp, scale=SCALE)
                            for t in range(2):
                                nc.tensor.matmul(
                                    psC[0:65, t * 512:(t + 1) * 512],
                                    v_sb[:, i, h, 0:65],
                                    expS[:, i, t * 512:(t + 1) * 512],
                                    start=(i == 0), stop=(i == KT - 1))
                        # reciprocal of rowsum without a table-set switch
                        lnr = p2.tile([1, N], F32, tag="lnr", bufs=1)
                        nc.scalar.activation(lnr[:, :], psC[64:65, :], AF.Ln)
                        recip = p2.tile([1, N], BF16, tag="recip", bufs=2)
                        nc.scalar.activation(recip[:, :], lnr[:, :], AF.Exp,
                                             scale=-1.0)
                        # broadcast recip over partitions with a K=1 matmul
                        psB = ps.tile([P, N], F32, tag="b", bufs=1)
                        for t in range(2):
                            nc.tensor.matmul(psB[:, t * 512:(t + 1) * 512],
                                             ones_row[:, :],
                                             recip[:, t * 512:(t + 1) * 512],
                                             start=True, stop=True)
                        bcast = p2.tile([P, N], BF16, tag="bcast", bufs=2)
                        nc.vector.tensor_copy(out=bcast[:], in_=psB[:, :])
                        # normalized attention rows -> DRAM (bf16; host upcasts)
                        for i in range(KT):
                            at = p2.tile([P, N], BF16, tag="attn", bufs=3)
                            nc.vector.tensor_mul(out=at[:], in0=expS[:, i, :],
                                                 in1=bcast[:])
                            nc.sync.dma_start(
                                out=attnT_d.ap()[h, i * P:(i + 1) * P, :],
                                in_=at[:])
                        # normalized context -> ctxT
                        nc.vector.tensor_mul(
                            out=ctxT_sb[hb:hb + 64, h // 2, :],
                            in0=psC[0:64, :], in1=bcast[0:64, :])

                    # ordering: unblock head 0 early, then interleave
                    qk_tile(0)
                    qk_tile(KT)
                    for m in range(KT):
                        v_tile(m)
                    for h in range(H):
                        if h > 0 and h % 2 == 0:
                            qk_tile(h // 2)
                            qk_tile(KT + h // 2)
                        head(h)

                    # ---- projection: outT = wp-tile.T @ ctxT + bias
                    for m in range(KT):
                        psA = ps.tile([P, N], F32, tag="s", bufs=2)
                        for k in range(KT):
                            for t in range(2):
                                nc.tensor.matmul(
                                    psA[:, t * 512:(t + 1) * 512],
                                    wp_sb[:, k, m * P:(m + 1) * P],
                                    ctxT_sb[:, k, t * 512:(t + 1) * 512],
                                    start=(k == 0), stop=(k == KT - 1))
                        ot = p2.tile([P, N], F32, tag="out", bufs=2)
                        nc.vector.tensor_scalar_add(ot[:], psA[:, :],
                                                    bp_sb[:, m:m + 1])
                        nc.sync.dma_start(out=outT_d.ap()[m * P:(m + 1) * P, :],
                                          in_=ot[:])

    nc.compile()
    return nc


def _in_maps(x, w_qkv, w_proj, b_proj):
    import ml_dtypes
    bf16 = ml_dtypes.bfloat16
    x = np.asarray(x, np.float32)
    w_qkv = np.asarray(w_qkv, np.float32)
    w_proj = np.asarray(w_proj, np.float32)
    b_proj = np.asarray(b_proj, np.float32)
    wqk = np.ascontiguousarray(w_qkv[:, :2 * C]).astype(bf16)
    wv = np.ascontiguousarray(w_qkv[:, 2 * C:]).astype(bf16)
    wp = w_proj.astype(bf16)
    bp = np.ascontiguousarray(b_proj.reshape(KT, P).T)
    return [{
        "xT": np.ascontiguousarray(x[b].T).astype(bf16),
        "wqk": wqk, "wv": wv, "wp": wp, "bp": bp,
    } for b in range(B)]


def _assemble(results):
    out = np.empty((B, N, C), np.float32)
    attn = np.empty((B, H, N, N), np.float32)
    for b in range(B):
        out[b] = results[b]["outT"].T
        attn[b] = np.asarray(results[b]["attnT"], np.float32).transpose(0, 2, 1)
    return out, attn


def kernel(x, w_qkv, w_proj, b_proj):
    from concourse.bass_utils import run_bass_kernel_spmd
    nc = _build()
    in_maps = _in_maps(x, w_qkv, w_proj, b_proj)
    res = run_bass_kernel_spmd(nc, in_maps, core_ids=list(range(NCORES)))
    return _assemble(res.results)


# revision 8
# speedup vs baseline: 1.0768x; 1.0768x over previous
"""Multi-head attention forward on 8 Trainium2 NeuronCores.

Problem: B=8, N=1024 tokens, C=1024 channels, H=16 heads, hd=64.
Returns (out [B,N,C], attn [B,H,N,N]) matching the reference
    qkv = x @ w_qkv ; attn = softmax(q k^T / sqrt(hd)) ; out = (attn v) @ w_proj + b_proj

Sharding: pure data parallel, one batch element per core, no collectives.

Per-core layout strategy (everything transpose-free on device):
  - host sends xT = x[b].T                       [C, N]   bf16
  - qT,kT = (w_qkv[:, :2C]).T-style matmul:      lhsT=w_qkv tile, rhs=xT  -> qkT [2C, N]
  - v     = x @ w_qkv[:, 2C:]:                   lhsT=xT tile, rhs=wv     -> v   [N(keys), C]
  - S^T_h [keys, tokens] = kT_h.T-matmul:        lhsT=kT_h (K=64), rhs=qT_h
  - expS = Exp(S^T * hd^-0.5)  (ACT, bf16 out; no max subtraction, scores are O(5))
  - ctx^T_h [64, N] = v_h.T @ expS  (accumulated over key tiles); a col-packed M=1
    ones-matmul accumulates rowsum[tokens] into psum partition 64 for free
  - recip = Exp(-Ln(rowsum))  (both funcs in the natural_log_exp table set)
  - bcast recip to 128 partitions with a K=1 ones matmul; attn^T = expS * bcast (DVE
    bf16 2x mode), DMA out per key tile; ctx^T normalized by the same bcast
  - out^T = w_proj-tile.T @ ctx^T + b_proj (per-partition bias), DMA out
Host transposes attnT -> attn and outT -> out (pure data movement).
"""

import numpy as np

B, N, C, H, HD = 8, 1024, 1024, 16, 64
P = 128
KT = C // P          # 8 tiles of 128 along any C/N axis
SCALE = HD ** -0.5   # 0.125
NCORES = 8


def _patch_act_tables():
    """Make Exp and Ln resolve to the single table set that contains both
    (natural_log_exp_and_others), so the kernel emits one ACT_TABLE_LOAD
    instead of thrashing between exp_and_others and the ln set per head."""
    import concourse.mybir as mybir
    import concourse.hw_specs as hw_specs
    from concourse import bacc as bacc_mod

    if getattr(hw_specs, "_attn_tables_patched", False):
        return
    AF = mybir.ActivationFunctionType
    orig = hw_specs.get_activation_tables

    def patched(arch):
        tabs = orig(arch)
        out = {}
        for name, funcs in tabs.items():
            if name == "natural_log_exp_and_others":
                out[name] = funcs
            else:
                out[name] = {f for f in funcs if f not in (AF.Exp, AF.Ln)}
        return out

    hw_specs.get_activation_tables = patched
    hw_specs._attn_tables_patched = True
    if getattr(bacc_mod, "get_activation_tables", None) is orig:
        bacc_mod.get_activation_tables = patched


def _build():
    import concourse.mybir as mybir
    import concourse.tile as tile
    from concourse import bacc

    _patch_act_tables()

    F32 = mybir.dt.float32
    BF16 = mybir.dt.bfloat16
    AF = mybir.ActivationFunctionType

    nc = bacc.Bacc("TRN2", target_bir_lowering=False, debug=False,
                   enable_asserts=True)

    xT_d = nc.dram_tensor("xT", [C, N], BF16, kind="ExternalInput")
    wqk_d = nc.dram_tensor("wqk", [C, 2 * C], BF16, kind="ExternalInput")
    wv_d = nc.dram_tensor("wv", [C, C], BF16, kind="ExternalInput")
    wp_d = nc.dram_tensor("wp", [C, C], BF16, kind="ExternalInput")
    bp_d = nc.dram_tensor("bp", [P, KT], F32, kind="ExternalInput")
    attnT_d = nc.dram_tensor("attnT", [H, N, N], BF16, kind="ExternalOutput")
    outT_d = nc.dram_tensor("outT", [C, N], F32, kind="ExternalOutput")

    with tile.TileContext(nc) as tc:
        with tc.tile_pool(name="persist", bufs=1) as pp, \
             tc.tile_pool(name="psum", bufs=1, space="PSUM") as ps:

            # ---- persistent SBUF tensors
            qk_sb = pp.tile([P, 2 * KT, N], BF16)   # outer 0..7 = qT, 8..15 = kT
            # v with a ones-column appended per head: cols 0..63 = v, 64..65 = 1.0
            # (col 64 rides along in the ctx matmul to accumulate softmax rowsums)
            v_sb = pp.tile([P, KT, H, 66], BF16)
            wp_sb = pp.tile([P, KT, C], BF16)
            bp_sb = pp.tile([P, KT], F32)
            ctxT_sb = pp.tile([P, KT, N], BF16)
            ones_row = pp.tile([1, P], BF16)

            nc.vector.memset(v_sb[:, :, :, 64:66], 1.0)
            nc.vector.memset(ones_row[:], 1.0)

            # keep the PE HAM clock-gate warm while input DMAs land: ~25us of
            # dummy matmuls on a zeroed tile (reuses the "b" psum slot, whose
            # first real use is much later)
            warm = pp.tile([P, 512], BF16)
            nc.vector.memset(warm[:], 0.0)
            wps = ps.tile([P, N], F32, tag="b", bufs=1)
            for _ in range(80):
                nc.tensor.matmul(wps[:, 0:512], warm[:, 0:P], warm[:, :],
                                 start=True, stop=True)
            nc.sync.dma_start(out=bp_sb[:], in_=bp_d.ap())
            for k in range(KT):
                nc.sync.dma_start(out=wp_sb[:, k, :],
                                  in_=wp_d.ap()[k * P:(k + 1) * P, :])

            with tc.tile_pool(name="ph1", bufs=1) as p1:
                xT_sb = p1.tile([P, KT, N], BF16)
                wqk_sb = p1.tile([P, KT, 2 * C], BF16)
                wv_sb = p1.tile([P, KT, C], BF16)
                for k in range(KT):
                    nc.sync.dma_start(out=xT_sb[:, k, :],
                                      in_=xT_d.ap()[k * P:(k + 1) * P, :])
                for mg in range(4):  # 512-col chunks so early m-tiles unblock fast
                    for k in range(KT):
                        nc.sync.dma_start(
                            out=wqk_sb[:, k, mg * 512:(mg + 1) * 512],
                            in_=wqk_d.ap()[k * P:(k + 1) * P, mg * 512:(mg + 1) * 512])
                for k in range(KT):
                    nc.sync.dma_start(out=wv_sb[:, k, :],
                                      in_=wv_d.ap()[k * P:(k + 1) * P, :])

                def qk_tile(m):
                    # qkT rows m*128..(m+1)*128 = (x @ w_qkv[:, :2C]).T tile
                    psA = ps.tile([P, N], F32, tag="s", bufs=2)
                    for k in range(KT):
                        for t in range(2):
                            nc.tensor.matmul(
                                psA[:, t * 512:(t + 1) * 512],
                                wqk_sb[:, k, m * P:(m + 1) * P],
                                xT_sb[:, k, t * 512:(t + 1) * 512],
                                start=(k == 0), stop=(k == KT - 1))
                    nc.vector.tensor_copy(out=qk_sb[:, m, :], in_=psA[:, :])

                def v_tile(m):
                    # v rows (keys) m*128..(m+1)*128 = (x @ w_qkv[:, 2C:]) tile
                    psA = ps.tile([P, N], F32, tag="s", bufs=2)
                    for k in range(KT):
                        for t in range(2):
                            nc.tensor.matmul(
                                psA[:, t * 512:(t + 1) * 512],
                                xT_sb[:, k, m * P:(m + 1) * P],
                                wv_sb[:, k, t * 512:(t + 1) * 512],
                                start=(k == 0), stop=(k == KT - 1))
                    nc.vector.tensor_copy(
                        out=v_sb[:, m, :, 0:64],
                        in_=psA[:, :].rearrange("p (h d) -> p h d", d=HD))

                with tc.tile_pool(name="ph2", bufs=1) as p2:

                    def head(h):
                        hb = (h % 2) * 64       # partition base of this head's dims
                        mq = h // 2             # qT outer index
                        mk = KT + h // 2        # kT outer index
                        expS = p2.tile([P, KT, N], BF16, tag="expS", bufs=2)
                        psC = ps.tile([P, N], F32, tag="c", bufs=1)
                        for i in range(KT):
                            psS = ps.tile([P, N], F32, tag="s", bufs=2)
                            for t in range(2):
                                nc.tensor.matmul(
                                    psS[:, t * 512:(t + 1) * 512],
                                    qk_sb[hb:hb + 64, mk, i * P:(i + 1) * P],
                                    qk_sb[hb:hb + 64, mq, t * 512:(t + 1) * 512],
                                    start=True, stop=True)
                            nc.scalar.activation(expS[:, i, :], psS[:, :],
                                                 AF.Exp, scale=SCALE)
                            for t in range(2):
                                nc.tensor.matmul(
                                    psC[0:65, t * 512:(t + 1) * 512],
                                    v_sb[:, i, h, 0:65],
                                    expS[:, i, t * 512:(t + 1) * 512],
                                    start=(i == 0), stop=(i == KT - 1))
                        # reciprocal of rowsum without a table-set switch
                        lnr = p2.tile([1, N], F32, tag="lnr", bufs=1)
                        nc.scalar.activation(lnr[:, :], psC[64:65, :], AF.Ln)
                        recip = p2.tile([1, N], BF16, tag="recip", bufs=2)
                        nc.scalar.activation(recip[:, :], lnr[:, :], AF.Exp,
                                             scale=-1.0)
                        # broadcast recip over partitions with a K=1 matmul
                        psB = ps.tile([P, N], F32, tag="b", bufs=1)
                        for t in range(2):
                            nc.tensor.matmul(psB[:, t * 512:(t + 1) * 512],
                                             ones_row[:, :],
                                             recip[:, t * 512:(t + 1) * 512],
                                             start=True, stop=True)
                        bcast = p2.tile([P, N], BF16, tag="bcast", bufs=2)
                        nc.vector.tensor_copy(out=bcast[:], in_=psB[:, :])
                        # normalized attention rows -> DRAM (bf16; host upcasts)
                        for i in range(KT):
                            at = p2.tile([P, N], BF16, tag="attn", bufs=3)
                            nc.vector.tensor_mul(out=at[:], in0=expS[:, i, :],
                                                 in1=bcast[:])
                            nc.sync.dma_start(
                                out=attnT_d.ap()[h, i * P:(i + 1) * P, :],
                                in_=at[:])
                        # normalized context -> ctxT
                        nc.vector.tensor_mul(
                            out=ctxT_sb[hb:hb + 64, h // 2, :],
                            in0=psC[0:64, :], in1=bcast[0:64, :])

                    # ordering: unblock head 0 early, then interleave
                    qk_tile(0)
                    qk_tile(KT)
                    for m in range(KT):
                        v_tile(m)
                    for h in range(H):
                        if h > 0 and h % 2 == 0:
                            qk_tile(h // 2)
                            qk_tile(KT + h // 2)
                        head(h)

                    # ---- projection: outT = wp-tile.T @ ctxT + bias
                    for m in range(KT):
                        psA = ps.tile([P, N], F32, tag="s", bufs=2)
                        for k in range(KT):
                            for t in range(2):
                                nc.tensor.matmul(
                                    psA[:, t * 512:(t + 1) * 512],
                                    wp_sb[:, k, m * P:(m + 1) * P],
                                    ctxT_sb[:, k, t * 512:(t + 1) * 512],
                                    start=(k == 0), stop=(k == KT - 1))
                        ot = p2.tile([P, N], F32, tag="out", bufs=2)
                        nc.vector.tensor_scalar_add(ot[:], psA[:, :],
                                                    bp_sb[:, m:m + 1])
                        nc.sync.dma_start(out=outT_d.ap()[m * P:(m + 1) * P, :],
                                          in_=ot[:])

    nc.compile()
    return nc


def _in_maps(x, w_qkv, w_proj, b_proj):
    import ml_dtypes
    bf16 = ml_dtypes.bfloat16
    x = np.asarray(x, np.float32)
    w_qkv = np.asarray(w_qkv, np.float32)
    w_proj = np.asarray(w_proj, np.float32)
    b_proj = np.asarray(b_proj, np.float32)
    wqk = np.ascontiguousarray(w_qkv[:, :2 * C]).astype(bf16)
    wv = np.ascontiguousarray(w_qkv[:, 2 * C:]).astype(bf16)
    wp = w_proj.astype(bf16)
    bp = np.ascontiguousarray(b_proj.reshape(KT, P).T)
    return [{
        "xT": np.ascontiguousarray(x[b].T).astype(bf16),
        "wqk": wqk, "wv": wv, "wp": wp, "bp": bp,
    } for b in range(B)]


def _assemble(results):
    out = np.empty((B, N, C), np.float32)
    attn = np.empty((B, H, N, N), np.float32)
    for b in range(B):
        out[b] = results[b]["outT"].T
        attn[b] = np.asarray(results[b]["attnT"], np.float32).transpose(0, 2, 1)
    return out, attn


def kernel(x, w_qkv, w_proj, b_proj):
    from concourse.bass_utils import run_bass_kernel_spmd
    nc = _build()
    in_maps = _in_maps(x, w_qkv, w_proj, b_proj)
    res = run_bass_kernel_spmd(nc, in_maps, core_ids=list(range(NCORES)))
    return _assemble(res.results)


# revision 11
# speedup vs baseline: 1.1611x; 1.0783x over previous
"""Multi-head attention forward on 8 Trainium2 NeuronCores.

Problem: B=8, N=1024 tokens, C=1024 channels, H=16 heads, hd=64.
Returns (out [B,N,C], attn [B,H,N,N]) matching the reference
    qkv = x @ w_qkv ; attn = softmax(q k^T / sqrt(hd)) ; out = (attn v) @ w_proj + b_proj

Sharding: pure data parallel, one batch element per core, no collectives.

Per-core layout strategy (everything transpose-free on device):
  - host sends xT = x[b].T                       [C, N]   bf16
  - qT,kT = (w_qkv[:, :2C]).T-style matmul:      lhsT=w_qkv tile, rhs=xT  -> qkT [2C, N]
  - v     = x @ w_qkv[:, 2C:]:                   lhsT=xT tile, rhs=wv     -> v   [N(keys), C]
  - S^T_h [keys, tokens] = kT_h.T-matmul:        lhsT=kT_h (K=64), rhs=qT_h
  - expS = Exp(S^T * hd^-0.5)  (ACT, bf16 out; no max subtraction, scores are O(5))
  - ctx^T_h [64, N] = v_h.T @ expS  (accumulated over key tiles); a col-packed M=1
    ones-matmul accumulates rowsum[tokens] into psum partition 64 for free
  - recip = Exp(-Ln(rowsum))  (both funcs in the natural_log_exp table set)
  - bcast recip to 128 partitions with a K=1 ones matmul; attn^T = expS * bcast (DVE
    bf16 2x mode), DMA out per key tile; ctx^T normalized by the same bcast
  - out^T = w_proj-tile.T @ ctx^T + b_proj (per-partition bias), DMA out
Host transposes attnT -> attn and outT -> out (pure data movement).
"""

import numpy as np

B, N, C, H, HD = 8, 1024, 1024, 16, 64
P = 128
KT = C // P          # 8 tiles of 128 along any C/N axis
SCALE = HD ** -0.5   # 0.125
NCORES = 8


def _patch_act_tables():
    """Make Exp and Ln resolve to the single table set that contains both
    (natural_log_exp_and_others), so the kernel emits one ACT_TABLE_LOAD
    instead of thrashing between exp_and_others and the ln set per head."""
    import concourse.mybir as mybir
    import concourse.hw_specs as hw_specs
    from concourse import bacc as bacc_mod

    if getattr(hw_specs, "_attn_tables_patched", False):
        return
    AF = mybir.ActivationFunctionType
    orig = hw_specs.get_activation_tables

    def patched(arch):
        tabs = orig(arch)
        out = {}
        for name, funcs in tabs.items():
            if name == "natural_log_exp_and_others":
                out[name] = funcs
            else:
                out[name] = {f for f in funcs if f not in (AF.Exp, AF.Ln)}
        return out

    hw_specs.get_activation_tables = patched
    hw_specs._attn_tables_patched = True
    if getattr(bacc_mod, "get_activation_tables", None) is orig:
        bacc_mod.get_activation_tables = patched


def _build():
    import concourse.mybir as mybir
    import concourse.tile as tile
    from concourse import bacc

    _patch_act_tables()

    F32 = mybir.dt.float32
    BF16 = mybir.dt.bfloat16
    AF = mybir.ActivationFunctionType

    nc = bacc.Bacc("TRN2", target_bir_lowering=False, debug=False,
                   enable_asserts=True)

    xT_d = nc.dram_tensor("xT", [C, N], BF16, kind="ExternalInput")
    wqk_d = nc.dram_tensor("wqk", [C, 2 * C], BF16, kind="ExternalInput")
    wv_d = nc.dram_tensor("wv", [C, C], BF16, kind="ExternalInput")
    wp_d = nc.dram_tensor("wp", [C, C], BF16, kind="ExternalInput")
    bp_d = nc.dram_tensor("bp", [P, KT], F32, kind="ExternalInput")
    attnT_d = nc.dram_tensor("attnT", [H, N, N], BF16, kind="ExternalOutput")
    outT_d = nc.dram_tensor("outT", [C, N], F32, kind="ExternalOutput")

    with tile.TileContext(nc) as tc:
        with tc.tile_pool(name="persist", bufs=1) as pp, \
             tc.tile_pool(name="psum", bufs=1, space="PSUM") as ps:

            # ---- persistent SBUF tensors
            qk_sb = pp.tile([P, 2 * KT, N], BF16)   # outer 0..7 = qT, 8..15 = kT
            # v with a ones-column appended per head: cols 0..63 = v, 64..65 = 1.0
            # (col 64 rides along in the ctx matmul to accumulate softmax rowsums)
            v_sb = pp.tile([P, KT, H, 66], BF16)
            wp_sb = pp.tile([P, KT, C], BF16)
            bp_sb = pp.tile([P, KT], F32)
            ctxT_sb = pp.tile([P, KT, N], BF16)
            ones_row = pp.tile([1, P], BF16)

            nc.vector.memset(v_sb[:, :, :, 64:66], 1.0)
            nc.vector.memset(ones_row[:], 1.0)

            # keep the PE HAM clock-gate warm while input DMAs land: ~25us of
            # dummy matmuls on a zeroed tile (reuses the "b" psum slot, whose
            # first real use is much later)
            warm = pp.tile([P, 512], BF16)
            nc.vector.memset(warm[:], 0.0)
            wps = ps.tile([P, N], F32, tag="b", bufs=1)
            for _ in range(80):
                nc.tensor.matmul(wps[:, 0:512], warm[:, 0:P], warm[:, :],
                                 start=True, stop=True)
            nc.sync.dma_start(out=bp_sb[:], in_=bp_d.ap())
            for k in range(KT):
                nc.sync.dma_start(out=wp_sb[:, k, :],
                                  in_=wp_d.ap()[k * P:(k + 1) * P, :])

            with tc.tile_pool(name="ph1", bufs=1) as p1:
                xT_sb = p1.tile([P, KT, N], BF16)
                wqk_sb = p1.tile([P, KT, 2 * C], BF16)
                wv_sb = p1.tile([P, KT, C], BF16)
                for k in range(KT):
                    nc.sync.dma_start(out=xT_sb[:, k, :],
                                      in_=xT_d.ap()[k * P:(k + 1) * P, :])
                for mg in range(4):  # 512-col chunks so early m-tiles unblock fast
                    for k in range(KT):
                        nc.sync.dma_start(
                            out=wqk_sb[:, k, mg * 512:(mg + 1) * 512],
                            in_=wqk_d.ap()[k * P:(k + 1) * P, mg * 512:(mg + 1) * 512])
                for k in range(KT):
                    nc.sync.dma_start(out=wv_sb[:, k, :],
                                      in_=wv_d.ap()[k * P:(k + 1) * P, :])

                def qk_tile(m):
                    # qkT rows m*128..(m+1)*128 = (x @ w_qkv[:, :2C]).T tile
                    psA = ps.tile([P, N], F32, tag="s", bufs=2)
                    for k in range(KT):
                        for t in range(2):
                            nc.tensor.matmul(
                                psA[:, t * 512:(t + 1) * 512],
                                wqk_sb[:, k, m * P:(m + 1) * P],
                                xT_sb[:, k, t * 512:(t + 1) * 512],
                                start=(k == 0), stop=(k == KT - 1))
                    # psum->sbuf copy on ScalarE: ACT is idle during phase 1
                    # and Copy is in the same table set as Exp/Ln
                    nc.scalar.copy(qk_sb[:, m, :], psA[:, :])

                def v_tile(m):
                    # v rows (keys) m*128..(m+1)*128 = (x @ w_qkv[:, 2C:]) tile
                    psA = ps.tile([P, N], F32, tag="s", bufs=2)
                    for k in range(KT):
                        for t in range(2):
                            nc.tensor.matmul(
                                psA[:, t * 512:(t + 1) * 512],
                                xT_sb[:, k, m * P:(m + 1) * P],
                                wv_sb[:, k, t * 512:(t + 1) * 512],
                                start=(k == 0), stop=(k == KT - 1))
                    nc.scalar.copy(
                        v_sb[:, m, :, 0:64],
                        psA[:, :].rearrange("p (h d) -> p h d", d=HD))

                with tc.tile_pool(name="ph2", bufs=1) as p2:

                    def head(h, fillers):
                        # fillers: list of thunks emitting dense PE work
                        # (qk/v tiles); interleaved into the keytile stream so
                        # the PE never idles long enough to drop its clock
                        hb = (h % 2) * 64       # partition base of this head's dims
                        mq = h // 2             # qT outer index
                        mk = KT + h // 2        # kT outer index
                        expS = p2.tile([P, KT, N], BF16, tag="expS", bufs=2)
                        psC = ps.tile([P, N], F32, tag="c", bufs=1)

                        def s_mm(i):
                            psS = ps.tile([P, N], F32, tag="s", bufs=2)
                            for t in range(2):
                                nc.tensor.matmul(
                                    psS[:, t * 512:(t + 1) * 512],
                                    qk_sb[hb:hb + 64, mk, i * P:(i + 1) * P],
                                    qk_sb[hb:hb + 64, mq, t * 512:(t + 1) * 512],
                                    start=True, stop=True)
                            nc.scalar.activation(expS[:, i, :], psS[:, :],
                                                 AF.Exp, scale=SCALE)

                        def ctx_mm(i):
                            for t in range(2):
                                nc.tensor.matmul(
                                    psC[0:65, t * 512:(t + 1) * 512],
                                    v_sb[:, i, h, 0:65],
                                    expS[:, i, t * 512:(t + 1) * 512],
                                    start=(i == 0), stop=(i == KT - 1))

                        # NOTE: emission order IS dependency order for Tile;
                        # filler j=i+2 must precede ctx_mm(i+2) (head 0's v
                        # tiles feed its own ctx matmuls)
                        s_mm(0)
                        s_mm(1)
                        for f in fillers[:2]:
                            f()
                        for i in range(KT):
                            ctx_mm(i)
                            if i + 2 < KT:
                                s_mm(i + 2)
                            if 2 + i < len(fillers):
                                fillers[2 + i]()
                        # reciprocal of rowsum without a table-set switch
                        lnr = p2.tile([1, N], F32, tag="lnr", bufs=1)
                        nc.scalar.activation(lnr[:, :], psC[64:65, :], AF.Ln)
                        recip = p2.tile([1, N], BF16, tag="recip", bufs=2)
                        nc.scalar.activation(recip[:, :], lnr[:, :], AF.Exp,
                                             scale=-1.0)
                        # broadcast recip over partitions with a K=1 matmul
                        psB = ps.tile([P, N], F32, tag="b", bufs=1)
                        for t in range(2):
                            nc.tensor.matmul(psB[:, t * 512:(t + 1) * 512],
                                             ones_row[:, :],
                                             recip[:, t * 512:(t + 1) * 512],
                                             start=True, stop=True)
                        bcast = p2.tile([P, N], BF16, tag="bcast", bufs=2)
                        nc.vector.tensor_copy(out=bcast[:], in_=psB[:, :])
                        # normalized context first: releases psC for next head
                        nc.vector.tensor_mul(
                            out=ctxT_sb[hb:hb + 64, h // 2, :],
                            in0=psC[0:64, :], in1=bcast[0:64, :])
                        # normalized attention rows -> DRAM (bf16; host upcasts)
                        for i in range(KT):
                            at = p2.tile([P, N], BF16, tag="attn", bufs=3)
                            nc.vector.tensor_mul(out=at[:], in0=expS[:, i, :],
                                                 in1=bcast[:])
                            nc.sync.dma_start(
                                out=attnT_d.ap()[h, i * P:(i + 1) * P, :],
                                in_=at[:])

                    # head 0 needs qk tiles 0/8 upfront; v tiles and the
                    # remaining qk tiles are interleaved into the heads as
                    # PE filler work (odd head h prefetches qk for head h+1)
                    qk_tile(0)
                    qk_tile(KT)
                    fillers_by_head = [[] for _ in range(H)]
                    for m in range(KT):
                        fillers_by_head[0].append(
                            lambda m=m: v_tile(m))
                    for h in range(1, H, 2):
                        mq_next = (h + 1) // 2
                        if mq_next < KT:
                            fillers_by_head[h].append(
                                lambda m=mq_next: qk_tile(m))
                            fillers_by_head[h].append(
                                lambda m=KT + mq_next: qk_tile(m))
                    for h in range(H):
                        head(h, fillers_by_head[h])

                    # ---- projection: outT = wp-tile.T @ ctxT + bias
                    for m in range(KT):
                        psA = ps.tile([P, N], F32, tag="s", bufs=2)
                        for k in range(KT):
                            for t in range(2):
                                nc.tensor.matmul(
                                    psA[:, t * 512:(t + 1) * 512],
                                    wp_sb[:, k, m * P:(m + 1) * P],
                                    ctxT_sb[:, k, t * 512:(t + 1) * 512],
                                    start=(k == 0), stop=(k == KT - 1))
                        ot = p2.tile([P, N], F32, tag="out", bufs=2)
                        nc.vector.tensor_scalar_add(ot[:], psA[:, :],
                                                    bp_sb[:, m:m + 1])
                        nc.sync.dma_start(out=outT_d.ap()[m * P:(m + 1) * P, :],
                                          in_=ot[:])

    nc.compile()
    return nc


def _in_maps(x, w_qkv, w_proj, b_proj):
    import ml_dtypes
    bf16 = ml_dtypes.bfloat16
    x = np.asarray(x, np.float32)
    w_qkv = np.asarray(w_qkv, np.float32)
    w_proj = np.asarray(w_proj, np.float32)
    b_proj = np.asarray(b_proj, np.float32)
    wqk = np.ascontiguousarray(w_qkv[:, :2 * C]).astype(bf16)
    wv = np.ascontiguousarray(w_qkv[:, 2 * C:]).astype(bf16)
    wp = w_proj.astype(bf16)
    bp = np.ascontiguousarray(b_proj.reshape(KT, P).T)
    return [{
        "xT": np.ascontiguousarray(x[b].T).astype(bf16),
        "wqk": wqk, "wv": wv, "wp": wp, "bp": bp,
    } for b in range(B)]


def _assemble(results):
    out = np.empty((B, N, C), np.float32)
    attn = np.empty((B, H, N, N), np.float32)
    for b in range(B):
        out[b] = results[b]["outT"].T
        attn[b] = np.asarray(results[b]["attnT"], np.float32).transpose(0, 2, 1)
    return out, attn


def kernel(x, w_qkv, w_proj, b_proj):
    from concourse.bass_utils import run_bass_kernel_spmd
    nc = _build()
    in_maps = _in_maps(x, w_qkv, w_proj, b_proj)
    res = run_bass_kernel_spmd(nc, in_maps, core_ids=list(range(NCORES)))
    return _assemble(res.results)
